# revision 17
# baseline (speedup 1.0000x reference)
"""Trainium2 Bass kernel for the DGL-JTNN tree-GRU encoder.

Only the bottom-up pass matters for the output (root readout); the down
phase is dead code.  Structure per core (8 trees, data-parallel over trees):

- Host precomputes input-independent vocab tables (pure weight transforms):
    ML[v]  = sigmoid(e_v @ Wz1 + bz) * tanh(e_v @ Wh1 + bh)   (leaf message)
  so the whole leaf z/h~ work collapses into one table gather.
- All graph gathers use dma_gather(transpose=True), which delivers gathered
  table rows feature-major straight into SBUF (no PE transposes, no PSUM
  staging, no copy-out).
- Levels are stored in "parity order": level l+1 columns are [left children
  of level-l order | right children].  Every pair reduction (s = m_l + m_r,
  arm feeds) is then a contiguous half-add: packed APs, DVE 2x mode.
- Everything on-chip is bf16 except PSUM accumulation and biases (f32).
"""

import os
import sys

import numpy as np

for _p in ("/opt/trn_rl_repo",):
    if os.path.isdir(_p) and _p not in sys.path:
        sys.path.insert(0, _p)

B, DEPTH, H, VOCAB = 64, 10, 128, 780
NPT = 2 ** (DEPTH + 1) - 1
NCORES = 8
T = B // NCORES  # trees per core

LN = {l: T * (1 << l) for l in range(DEPTH + 1)}  # cols per level per core
CH = 1024  # chunk width (mm moving / ACT / TT)

# small levels packed into one gather stream: level -> offset in XS tile
_SMALL_LEVELS = [5, 4, 3, 2, 1, 0]
XS_OFF = {}
_o = 0
for _l in _SMALL_LEVELS:
    XS_OFF[_l] = _o
    _o += LN[_l]
XS_COLS = 512  # 504 used + 8 pad
assert _o <= XS_COLS

# gidx stream layout (int16 indices, wrapped %16):  [leaf | X9 | X8 | X7 | X6 | XS]
_GBLOCKS = [("leaf", LN[10]), ("x9", LN[9]), ("x8", LN[8]), ("x7", LN[7]),
            ("x6", LN[6]), ("xs", XS_COLS)]
GOFF = {}
_o = 0
for _nm, _sz in _GBLOCKS:
    GOFF[_nm] = _o
    _o += _sz
GTOT = _o           # 16384 idxs
GCOLS = GTOT // 16  # 1024 int16 cols

_NC_CACHE = {}

# engine for rm TT: pool | dve.  Pool is FIFO with the gathers, so putting
# rm there stalls the whole pipeline behind the gather queue -> use DVE.
RM_ENG = os.environ.get("DGLJ_RM_ENG", "dve")
LEAF_RM_ENG = os.environ.get("DGLJ_LEAF_RM_ENG", "dve")
# levels <= this use the latency-optimized t1-form chain
SMALL_MAX = int(os.environ.get("DGLJ_SMALL_MAX", "6"))


def _pair_order(nch):
    """Process chunks so consumers (which need chunk c and c + nch/2 of this
    level) unblock after two producer chunks."""
    if nch <= 1:
        return list(range(nch))
    h = nch // 2
    out = []
    for i in range(h):
        out += [i, h + i]
    return out


def _parity_patterns():
    """pat[l] = heap ids of level-l nodes in parity order.  Column k of level
    l is (pattern p = k // T, tree t = k % T); left child of col k lives at
    col k of level l+1, right child at col k + LN[l]."""
    pat = {0: [0]}
    for l in range(DEPTH):
        pl = pat[l]
        pat[l + 1] = [2 * h + 1 for h in pl] + [2 * h + 2 for h in pl]
    return pat


_PAT = _parity_patterns()


def _build_nc(reps=1):
    from contextlib import ExitStack

    import concourse.bass as bass
    import concourse.mybir as mybir
    import concourse.tile as tile
    from concourse import bacc

    f32 = mybir.dt.float32
    bf16 = mybir.dt.bfloat16
    i16 = mybir.dt.int16
    AF = mybir.ActivationFunctionType

    nc = bacc.Bacc("TRN2", target_bir_lowering=False)

    emb16_d = nc.dram_tensor("emb16", [VOCAB, H], bf16, kind="ExternalInput")
    mltab_d = nc.dram_tensor("mltab", [VOCAB, H], bf16, kind="ExternalInput")
    gidx_d = nc.dram_tensor("gidx", [128, GCOLS], i16, kind="ExternalInput")
    wpack16_d = nc.dram_tensor("wpack16", [H, 9 * H], bf16, kind="ExternalInput")
    wbias_d = nc.dram_tensor("wbias", [H, 4], f32, kind="ExternalInput")
    out_d = nc.dram_tensor("out", [H, T], f32, kind="ExternalOutput")

    _W = ("wz1", "wz2", "wh1", "wh2", "wr", "ur", "wg1", "wg2", "nur")
    _B = ("bz", "bh", "br", "bg")

    with tile.TileContext(nc) as tc, ExitStack() as ctx:
        consts = ctx.enter_context(tc.tile_pool(name="consts", bufs=1))
        xpool = ctx.enter_context(tc.tile_pool(name="xp", bufs=1))
        mpool = ctx.enter_context(tc.tile_pool(name="mp", bufs=1))
        ck = ctx.enter_context(tc.tile_pool(name="ck", bufs=3))
        pzp = ctx.enter_context(tc.tile_pool(name="pz", bufs=2, space="PSUM"))
        php = ctx.enter_context(tc.tile_pool(name="ph", bufs=2, space="PSUM"))

        # ---- constants ----
        # gidx split: leaf block first so leaf gathers start ASAP
        gidx = consts.tile([128, GCOLS], i16, tag="gidx", name="gidx")
        lcols = GOFF["x9"] // 16  # leaf block cols
        wbias = consts.tile([H, 4], f32, tag="wb", name="wbias")
        nc.sync.dma_start(out=wbias[:], in_=wbias_d[:])
        nc.sync.dma_start(out=gidx[:, :lcols], in_=gidx_d[:, :lcols])
        w16 = consts.tile([H, 9 * H], bf16, tag="w16", name="w16")
        nc.sync.dma_start(out=w16[:], in_=wpack16_d[:])
        nc.sync.dma_start(out=gidx[:, lcols:], in_=gidx_d[:, lcols:])
        wsb = {n: w16[:, i * H : (i + 1) * H] for i, n in enumerate(_W)}
        bsb = {n: wbias[:, i : i + 1] for i, n in enumerate(_B)}
        # dummy 1-col sigmoid: hoist ACT table load into startup
        warm = consts.tile([H, 1], f32, tag="warm", name="warm")
        nc.scalar.activation(warm[:], wbias[:, :1], AF.Sigmoid)

        MM = 512  # max moving cols per matmul writing PSUM (one bank)

        def accum(psum, w, terms):
            """Accumulate sum of (weight_ap, rhs_fn) into psum[:, :w].

            rhs_fn(s0, sw) returns the moving operand for psum cols
            [s0, s0+sw).  Emitted in 512-col segments (PSUM bank limit),
            each segment its own accumulation group."""
            for s0 in range(0, w, MM):
                sw = min(MM, w - s0)
                for ti, (wap, rhs_fn) in enumerate(terms):
                    nc.tensor.matmul(
                        psum[:, s0 : s0 + sw], wap, rhs_fn(s0, sw),
                        start=(ti == 0), stop=(ti == len(terms) - 1),
                    )

        def gather(dst3, src_d, goff, i0, cnt):
            """Gather cnt rows of src_d (feature-major into dst3[:, :, i0:i0+cnt])
            using gidx block at goff+i0.  Transpose-mode gathers are limited
            to 512 indices per instruction (1024 crashes the device)."""
            for s0 in range(i0, i0 + cnt, 512):
                sc = min(512, i0 + cnt - s0)
                c0 = (goff + s0) // 16
                nc.gpsimd.dma_gather(
                    dst3[:, :, s0 : s0 + sc],
                    src_d[:, :],
                    gidx[:, c0 : c0 + sc // 16],
                    num_idxs=sc,
                    num_idxs_reg=sc,
                    elem_size=H,
                    transpose=True,
                )

        # ---- X / table tiles (3D: [128, 1, n] so gather APs line up) ----
        ML3 = xpool.tile([128, 1, LN[10]], bf16, tag="ml", name="ML")
        X9_3 = xpool.tile([128, 1, LN[9]], bf16, tag="x9", name="X9")
        X8_3 = xpool.tile([128, 1, LN[8]], bf16, tag="x8", name="X8")
        X7_3 = xpool.tile([128, 1, LN[7]], bf16, tag="x7", name="X7")
        X6_3 = xpool.tile([128, 1, LN[6]], bf16, tag="x6", name="X6")
        XS_3 = xpool.tile([128, 1, XS_COLS], bf16, tag="xs", name="XS")
        ML = ML3[:, 0, :]

        def xview(l):
            if l == 9:
                return X9_3[:, 0, :]
            if l == 8:
                return X8_3[:, 0, :]
            if l == 7:
                return X7_3[:, 0, :]
            if l == 6:
                return X6_3[:, 0, :]
            return XS_3[:, 0, XS_OFF[l] : XS_OFF[l] + LN[l]]

        for _rep in range(reps):
            # ---- gathers (Pool, FIFO): feed leaf chunks in pair order ----
            lorder = _pair_order(LN[10] // CH)  # leaf chunk order
            for i, c in enumerate(lorder):
                gather(ML3, mltab_d, GOFF["leaf"], c * CH, CH)
                if i % 2 == 1 and i // 2 < 4:
                    x = (i // 2) * CH
                    gather(X9_3, emb16_d, GOFF["x9"], x, CH)
            for c in range(2):
                gather(X8_3, emb16_d, GOFF["x8"], c * CH, CH)
            gather(X7_3, emb16_d, GOFF["x7"], 0, LN[7])
            gather(X6_3, emb16_d, GOFF["x6"], 0, LN[6])
            gather(XS_3, emb16_d, GOFF["xs"], 0, XS_COLS)

            # ---- leaf r/rm:  rm = sigmoid(Wr@x_p + Ur@ML + br) * ML ----
            RM10 = mpool.tile([128, LN[10]], bf16, tag="rm0", name="RM10")
            X9 = xview(9)
            for i, c in enumerate(lorder):
                c0 = c * CH
                pr = (pzp if i % 2 == 0 else php).tile(
                    [128, CH], f32, tag="pz" if i % 2 == 0 else "ph", name=f"lpr{c}"
                )
                x0 = c0 % LN[9]
                accum(pr, CH, [
                    (wsb["wr"], lambda s0, sw: X9[:, x0 + s0 : x0 + s0 + sw]),
                    (wsb["ur"], lambda s0, sw: ML[:, c0 + s0 : c0 + s0 + sw]),
                ])
                r = ck.tile([128, CH], bf16, tag="r", name=f"lr{c}")
                nc.scalar.activation(r[:], pr[:], AF.Sigmoid, bias=bsb["br"])
                eng = nc.gpsimd if LEAF_RM_ENG == "pool" else nc.vector
                eng.tensor_mul(RM10[:, c0 : c0 + CH], r[:], ML[:, c0 : c0 + CH])

            # ---- levels 9..1 ----
            Mn, RMn = ML, RM10
            M1 = None
            for l in range(9, 0, -1):
                n = LN[l]
                X = xview(l)
                Xp = xview(l - 1)
                hp = LN[l - 1]  # parent level size = n // 2
                M = mpool.tile([128, n], bf16, tag=f"m{l % 2}", name=f"M{l}")
                RM = None
                if l >= 2:
                    RM = mpool.tile([128, n], bf16, tag=f"rm{l % 2}", name=f"RM{l}")
                S = mpool.tile([128, n], bf16, tag=f"s{l % 2}", name=f"S{l}")
                lch = CH if l == 9 else min(512, n)  # narrower chunks in the tail
                nch = (n + lch - 1) // lch
                small = l <= SMALL_MAX
                for c in _pair_order(nch):
                    c0 = c * lch
                    w = min(lch, n - c0)
                    # s = m_left + m_right (contiguous halves in parity order);
                    # Pool is idle once gathers drain, so put S there
                    nc.gpsimd.tensor_add(
                        S[:, c0 : c0 + w], Mn[:, c0 : c0 + w], Mn[:, n + c0 : n + c0 + w]
                    )
                    # z = sigmoid(Wz1@x + Wz2@s + bz)
                    pz = pzp.tile([128, CH], f32, tag="pz", name=f"pz{l}_{c}")
                    accum(pz, w, [
                        (wsb["wz1"], lambda s0, sw: X[:, c0 + s0 : c0 + s0 + sw]),
                        (wsb["wz2"], lambda s0, sw: S[:, c0 + s0 : c0 + s0 + sw]),
                    ])
                    z = ck.tile([128, CH], bf16, tag="z", name=f"z{l}_{c}")
                    nc.scalar.activation(z[:, :w], pz[:, :w], AF.Sigmoid, bias=bsb["bz"])
                    # h~ = tanh(Wh1@x + Wh2@rm_l + Wh2@rm_r + bh)
                    ph = php.tile([128, CH], f32, tag="ph", name=f"ph{l}_{c}")
                    accum(ph, w, [
                        (wsb["wh1"], lambda s0, sw: X[:, c0 + s0 : c0 + s0 + sw]),
                        (wsb["wh2"], lambda s0, sw: RMn[:, c0 + s0 : c0 + s0 + sw]),
                        (wsb["wh2"], lambda s0, sw: RMn[:, n + c0 + s0 : n + c0 + s0 + sw]),
                    ])
                    ht = ck.tile([128, CH], bf16, tag="h", name=f"ht{l}_{c}")
                    nc.scalar.activation(ht[:, :w], ph[:, :w], AF.Tanh, bias=bsb["bh"])
                    rmeng = nc.gpsimd if (RM_ENG == "pool" or not small) else nc.vector
                    if not small:
                        # m = s + z*(h~ - s)
                        u = ck.tile([128, CH], bf16, tag="u", name=f"u{l}_{c}")
                        nc.vector.tensor_sub(u[:, :w], ht[:, :w], S[:, c0 : c0 + w])
                        v = ck.tile([128, CH], bf16, tag="v", name=f"v{l}_{c}")
                        nc.vector.tensor_mul(v[:, :w], z[:, :w], u[:, :w])
                        nc.vector.tensor_add(M[:, c0 : c0 + w], S[:, c0 : c0 + w], v[:, :w])
                        if l >= 2:
                            # r = sigmoid(Wr@x_parent + Ur@m + br); rm = r*m
                            pr = (php if c % 2 == 0 else pzp).tile(
                                [128, CH], f32, tag="ph" if c % 2 == 0 else "pz",
                                name=f"pr{l}_{c}"
                            )
                            def xp_rhs(s0, sw):
                                p0 = (c0 + s0) % hp
                                return Xp[:, p0 : p0 + sw]
                            accum(pr, w, [
                                (wsb["wr"], xp_rhs),
                                (wsb["ur"], lambda s0, sw: M[:, c0 + s0 : c0 + s0 + sw]),
                            ])
                            r = ck.tile([128, CH], bf16, tag="r", name=f"r{l}_{c}")
                            nc.scalar.activation(r[:, :w], pr[:, :w], AF.Sigmoid,
                                                 bias=bsb["br"])
                            rmeng.tensor_mul(RM[:, c0 : c0 + w], r[:, :w],
                                             M[:, c0 : c0 + w])
                    else:
                        # latency form: t1=z*ht, t2=z*s, m=(s-t2)+t1,
                        # r-pre = Wr@xp + Ur@s - Ur@t2 + Ur@t1  (Ur@t1 last)
                        pr = None
                        if l >= 2:
                            pr = (php if c % 2 == 0 else pzp).tile(
                                [128, CH], f32, tag="ph" if c % 2 == 0 else "pz",
                                name=f"pr{l}_{c}"
                            )
                            # x-parent term first (earliest available)
                            if w <= hp:
                                nc.tensor.matmul(pr[:, :w], wsb["wr"], Xp[:, :w],
                                                 start=True, stop=False)
                            else:
                                nc.tensor.matmul(pr[:, :hp], wsb["wr"], Xp[:, :hp],
                                                 start=True, stop=False)
                                nc.tensor.matmul(pr[:, hp:w], wsb["wr"], Xp[:, :hp],
                                                 start=True, stop=False)
                            nc.tensor.matmul(pr[:, :w], wsb["ur"], S[:, c0 : c0 + w],
                                             start=False, stop=False)
                        t2 = ck.tile([128, CH], bf16, tag="u", name=f"t2{l}_{c}")
                        nc.vector.tensor_mul(t2[:, :w], z[:, :w], S[:, c0 : c0 + w])
                        mp = ck.tile([128, CH], bf16, tag="v", name=f"mp{l}_{c}")
                        nc.vector.tensor_sub(mp[:, :w], S[:, c0 : c0 + w], t2[:, :w])
                        if pr is not None:
                            nc.tensor.matmul(pr[:, :w], wsb["nur"], t2[:, :w],
                                             start=False, stop=False)
                        t1 = ck.tile([128, CH], bf16, tag="t1", name=f"t1{l}_{c}")
                        nc.vector.tensor_mul(t1[:, :w], z[:, :w], ht[:, :w])
                        nc.vector.tensor_add(M[:, c0 : c0 + w], mp[:, :w], t1[:, :w])
                        if pr is not None:
                            nc.tensor.matmul(pr[:, :w], wsb["ur"], t1[:, :w],
                                             start=False, stop=True)
                            r = ck.tile([128, CH], bf16, tag="r", name=f"r{l}_{c}")
                            nc.scalar.activation(r[:, :w], pr[:, :w], AF.Sigmoid,
                                                 bias=bsb["br"])
                            rmeng.tensor_mul(RM[:, c0 : c0 + w], r[:, :w],
                                             M[:, c0 : c0 + w])
                Mn, RMn = M, RM
                if l == 1:
                    M1 = M

            # ---- root readout: relu(Wg1@x_root + Wg2@(m_l + m_r) + bg) ----
            S0 = mpool.tile([128, T], bf16, tag="s0", name="S0")
            nc.vector.tensor_add(S0[:], M1[:, :T], M1[:, T : 2 * T])
            pg = pzp.tile([128, CH], f32, tag="pz", name="pg")
            nc.tensor.matmul(pg[:, :T], wsb["wg1"], xview(0), start=True, stop=False)
            nc.tensor.matmul(pg[:, :T], wsb["wg2"], S0[:], start=False, stop=True)
            outt = ck.tile([128, T], f32, tag="o", name="outt")
            nc.scalar.activation(outt[:], pg[:, :T], AF.Relu, bias=bsb["bg"])
            nc.sync.dma_start(out=out_d[:, :], in_=outt[:])

    nc.finalize()
    return nc


def get_nc(reps=1):
    key = ("nc", reps)
    if key not in _NC_CACHE:
        _NC_CACHE[key] = _build_nc(reps)
    return _NC_CACHE[key]


def _wrap_idx(ids):
    """int16 index stream -> [16, n/16] wrapped layout."""
    return ids.astype(np.int16).reshape(-1, 16).T


def make_core_inputs(wid, emb, weights):
    import ml_dtypes

    bf16 = ml_dtypes.bfloat16
    e = np.asarray(emb, dtype=np.float32)
    Wz, Wh = np.asarray(weights["Wz_w"], np.float32), np.asarray(weights["Wh_w"], np.float32)
    Wr, Ur = np.asarray(weights["Wr_w"], np.float32), np.asarray(weights["Ur_w"], np.float32)
    Wg = np.asarray(weights["Wg_w"], np.float32)
    bz, bh = np.asarray(weights["Wz_b"], np.float32), np.asarray(weights["Wh_b"], np.float32)
    br, bg = np.asarray(weights["Ur_b"], np.float32), np.asarray(weights["Wg_b"], np.float32)

    # vocab tables (input-independent weight transforms)
    zl = 1.0 / (1.0 + np.exp(-(e @ Wz[:H] + bz)))
    hl = np.tanh(e @ Wh[:H] + bh)
    ML32 = (zl * hl).astype(np.float32)
    mltab = np.ascontiguousarray(ML32.astype(bf16))
    emb16 = np.ascontiguousarray(e.astype(bf16))

    wmats = [Wz[:H], Wz[H:], Wh[:H], Wh[H:], Wr, Ur, Wg[:H], Wg[H:], -Ur]
    wpack16 = np.concatenate(wmats, axis=1).astype(bf16)
    wbias = np.stack([bz, bh, br, bg], axis=1).astype(np.float32)

    base = {
        "emb16": emb16,
        "mltab": mltab,
        "wpack16": np.ascontiguousarray(wpack16),
        "wbias": np.ascontiguousarray(wbias),
    }

    wid = np.asarray(wid).reshape(B, NPT)
    in_maps = []
    for cid in range(NCORES):
        widc = wid[cid * T : (cid + 1) * T]  # [T, NPT]
        blocks = []
        # leaf + x9..x6 + xs, each level's wids in parity (pattern-major) order
        def lvl_ids(l):
            pat = np.asarray(_PAT[l])
            return widc[np.arange(T)[None, :], pat[:, None]].ravel()  # [P*T]

        blocks.append(lvl_ids(10))
        for l in (9, 8, 7, 6):
            blocks.append(lvl_ids(l))
        xs = np.concatenate([lvl_ids(l) for l in _SMALL_LEVELS])
        xs = np.concatenate([xs, np.zeros(XS_COLS - len(xs), np.int64)])
        blocks.append(xs)
        ids = np.concatenate(blocks)
        assert len(ids) == GTOT
        gi = _wrap_idx(ids)  # [16, GCOLS]
        in_maps.append({**base, "gidx": np.ascontiguousarray(np.tile(gi, (8, 1)))})
    return in_maps


def kernel(**inputs):
    from concourse.bass_utils import run_bass_kernel_spmd

    nc = get_nc()
    in_maps = make_core_inputs(inputs["wid"], inputs["emb"], inputs)
    res = run_bass_kernel_spmd(nc, in_maps, core_ids=list(range(NCORES)))
    out = np.concatenate(
        [np.asarray(res.results[c]["out"]).T for c in range(NCORES)], axis=0
    )
    return np.ascontiguousarray(out.astype(np.float32))
